# revision 20
# baseline (speedup 1.0000x reference)
"""Trainium2 Bass kernel for nn_AttnDecoderRNN (GRU step + Bahdanau attention
over ragged sequences + BatchNorm MLP head).

Self-contained: hardcodes shapes (H=512, B=256, S=512, P=3) and the
data-parallel sharding (batch dim split across 8 cores).

Strategy
--------
Batch-parallel over 8 cores (32 batches each). Host bin-packs batches so every
core receives the same multiset of per-batch s-tile counts (ceil(len/128),
with a few batches promoted so class counts divide by 8); the device program
is compiled specialized to that static schedule, so invalid s-tiles are never
loaded or computed.

Per core: GRU runs in transposed [H, batch] layout on PE. Per batch: enc tiles
are DMA-cast fp32->bf16, PE-transposed to put H on partitions, multiplied with
We.T in bf16, sigmoid applied on ACT with the per-batch hidden projection as a
per-partition bias, reduced against Wv by matmul, then a grouped masked
softmax (fused exp+row-sum) and a context matmul against the natural-layout
enc tiles. The output MLP (W1+ReLU) runs on-device; BatchNorm statistics
(which couple all 256 batches) and the tiny 512->3 output matmul finish on
host.
"""

import sys
from contextlib import ExitStack

sys.path.insert(0, "/opt/trn_rl_repo")

import numpy as np
import ml_dtypes

import concourse.bacc as bacc
import concourse.tile as tile
from concourse import mybir
from concourse.masks import make_identity
from concourse.bass_utils import run_bass_kernel_spmd

F32 = mybir.dt.float32
BF16 = mybir.dt.bfloat16
AF = mybir.ActivationFunctionType

H = 512
B = 256
S = 512
P = 3
EPS = 1e-5
NCORES = 8
BC = B // NCORES          # 32 batches per core
TS = 128                  # s-tile size
NT_MAX = S // TS          # 4 tiles max per batch
GROUP = 8                 # batches per softmax group
NEG = -30000.0            # additive mask value (exp -> exact 0)

bf16 = ml_dtypes.bfloat16


# ----------------------------------------------------------------------------
# Host-side planning
# ----------------------------------------------------------------------------

def _plan(lengths: np.ndarray):
    """Assign batches to cores so each core gets an identical multiset of
    s-tile counts. Returns (perm[8][32] original batch idx, tcnt[32])."""
    lengths = np.asarray(lengths).astype(np.int64)
    cls = np.clip((lengths + TS - 1) // TS, 1, NT_MAX).astype(np.int64)
    # promote batches so each class count is divisible by NCORES
    by_cls = {k: [int(i) for i in np.nonzero(cls == k)[0]] for k in range(1, 5)}
    for k in range(1, 4):
        r = len(by_cls[k]) % NCORES
        if r:
            moved = sorted(by_cls[k], key=lambda i: lengths[i])[-r:]
            for i in moved:
                by_cls[k].remove(i)
                by_cls[k + 1].append(i)
    assert len(by_cls[4]) % NCORES == 0
    per_core_cls = {k: len(by_cls[k]) // NCORES for k in range(1, 5)}

    # per-core slot schedule: deal classes (desc) round-robin into the softmax
    # groups so per-group tile counts stay balanced
    tcnt = []
    for k in (1, 2, 3, 4):
        tcnt += [k] * per_core_cls[k]
    assert len(tcnt) == BC

    perm = [[None] * BC for _ in range(NCORES)]
    for c in range(NCORES):
        chunk = {k: by_cls[k][c * per_core_cls[k]:(c + 1) * per_core_cls[k]]
                 for k in range(1, 5)}
        ptr = {k: 0 for k in range(1, 5)}
        for slot, k in enumerate(tcnt):
            perm[c][slot] = chunk[k][ptr[k]]
            ptr[k] += 1
    return perm, tcnt


# ----------------------------------------------------------------------------
# Device program
# ----------------------------------------------------------------------------

def build_program(tcnt, reps: int = 1):
    """Build + compile the per-core Bass program for the given (static)
    per-slot tile counts. Returns finalized nc."""
    import concourse.bass as bass

    nc = bacc.Bacc("TRN2", target_bir_lowering=False, debug=False,
                   num_devices=NCORES)

    def din(name, shape, dt):
        return nc.dram_tensor(name, shape, dt, kind="ExternalInput")

    def dout(name, shape, dt):
        return nc.dram_tensor(name, shape, dt, kind="ExternalOutput")

    enc = din("enc", [S, BC, H], BF16)
    maskadd = din("maskadd", [BC, S], F32)
    palT = din("palT", [P, BC], BF16)
    lcTb = din("lcTb", [H, BC], BF16)
    lhTb = din("lhTb", [H, BC], BF16)
    lhT = din("lhT", [H, BC], F32)
    WeT = din("WeT", [H, H], BF16)
    Wv4 = din("Wv4", [128, 4], BF16)
    WihPT = din("WihPT", [P, 3 * H], BF16)
    WihCT = din("WihCT", [H, 3 * H], BF16)
    WhhT = din("WhhT", [H, 3 * H], BF16)
    WhT = din("WhT", [H, H], F32)
    W1T = din("W1T", [2 * H, H], F32)
    biases = din("biases", [128, 24], F32)

    hT_out = dout("hT_out", [H, BC], F32)
    ctx_out = dout("ctx_out", [BC, H], F32)
    yT_out = dout("yT_out", [H, BC], F32)
    aw_out = dout("aw_out", [BC, S], F32)

    NT = sum(tcnt)
    n_cls = {k: sum(1 for c in tcnt if c == k) for k in range(1, 5)}

    def ktiled(handle, nk, width, dt_size):
        """DRAM AP view [128, nk, width] of a [128*nk, width] tensor."""
        return bass.AP(
            tensor=handle, offset=0,
            ap=[[width, 128], [128 * width, nk], [1, width]])

    with tile.TileContext(nc) as tc, ExitStack() as ctx:
        consts = ctx.enter_context(tc.tile_pool(name="consts", bufs=1))

        ident_bf = consts.tile([128, 128], BF16)
        make_identity(nc, ident_bf)
        ident_f = consts.tile([128, 128], F32)
        make_identity(nc, ident_f)

        WeT_sb = consts.tile([128, 4, H], BF16)
        Wv_sb = consts.tile([128, 4], BF16)
        nc.sync.dma_start(out=Wv_sb, in_=Wv4.ap())
        WihPT_sb = consts.tile([P, 3 * H], BF16)
        nc.sync.dma_start(out=WihPT_sb, in_=WihPT.ap())
        lcTb_sb = consts.tile([128, 4, BC], BF16)
        nc.sync.dma_start(out=lcTb_sb, in_=ktiled(lcTb, 4, BC, 2))
        lhTb_sb = consts.tile([128, 4, BC], BF16)
        nc.sync.dma_start(out=lhTb_sb, in_=ktiled(lhTb, 4, BC, 2))
        WhT_sb = consts.tile([128, 4, H], F32)
        bias_sb = consts.tile([128, 24], F32)
        nc.sync.dma_start(out=bias_sb, in_=biases.ap())
        brz_sb = bias_sb[:, 0:8]
        bin_sb = bias_sb[:, 8:12]
        bhn_sb = bias_sb[:, 12:16]
        bebh_sb = bias_sb[:, 16:20]
        b1_sb = bias_sb[:, 20:24]
        palT_sb = consts.tile([P, BC], BF16)
        nc.sync.dma_start(out=palT_sb, in_=palT.ap())
        lhT_sb = consts.tile([128, 4, BC], F32)
        nc.sync.dma_start(out=lhT_sb, in_=ktiled(lhT, 4, BC, 4))

        persist = ctx.enter_context(tc.tile_pool(name="persist", bufs=1))
        h_sb = persist.tile([128, 4, BC], F32)        # h_new.T
        hid_sb = persist.tile([128, 4, BC], F32)      # Wh@h_new + be + bh
        gi_sb = persist.tile([128, 12, BC], F32)      # W_ih @ x.T
        energy_sb = persist.tile([BC, S], F32)
        aw_sb = persist.tile([BC, S], F32)
        ctxT_sb = persist.tile([128, 4, BC], F32)
        y_sb = persist.tile([128, 4, BC], F32)
        stat_sb = persist.tile([BC, 3], F32)          # negmax | sumexp | recip

        # PSUM: mm (2) + tr (2) + row (2) + awt (2) = 8 banks
        mm_ps = ctx.enter_context(tc.tile_pool(name="mm_ps", bufs=2, space="PSUM"))
        tr_ps = ctx.enter_context(tc.tile_pool(name="tr_ps", bufs=4, space="PSUM"))
        row_ps = ctx.enter_context(tc.tile_pool(name="row_ps", bufs=2, space="PSUM"))
        awt_ps_pool = tr_ps

        small = ctx.enter_context(tc.tile_pool(name="small", bufs=4))
        # one 24KB/partition slot reused: WihCT -> WhhT -> W1T
        gruw = ctx.enter_context(tc.tile_pool(name="gruw", bufs=2))
        enc_pool = ctx.enter_context(tc.tile_pool(name="enc_pool", bufs=1))
        encT_pool = ctx.enter_context(tc.tile_pool(name="encT_pool", bufs=4))
        sig_pool = ctx.enter_context(tc.tile_pool(name="sig_pool", bufs=3))
        aw_pool = ctx.enter_context(tc.tile_pool(name="aw_pool", bufs=2))
        rows = ctx.enter_context(tc.tile_pool(name="rows", bufs=4))
        dram = ctx.enter_context(tc.tile_pool(name="dram", bufs=1, space="DRAM"))

        def enc_batch_ap(j, nt):
            # [128 p(s%128), nt t(s//128), H] view of enc[:, j, :]
            return bass.AP(
                tensor=enc, offset=j * H,
                ap=[[BC * H, 128], [TS * BC * H, nt], [1, H]])

        def body(_iv=None):
            PRE = 4
            enc_tiles = {}
            encTs = {}

            def _load0(j):
                nt = tcnt[j]
                et = enc_pool.tile([128, nt, H], BF16, tag=f"enc{nt}",
                                   bufs=n_cls[nt], name="et")
                nc.sync.dma_start(out=et, in_=enc_batch_ap(j, nt))
                enc_tiles[j] = et

            def transpose(j):
                nt = tcnt[j]
                L = nt * TS
                et = enc_tiles[j]
                encT = encT_pool.tile([128, 4, S], BF16, tag="encT",
                                      name="encT")
                for k in range(4):
                    trp = tr_ps.tile([128, S], BF16, tag="trp", name="trp")
                    for t in range(nt):
                        nc.tensor.transpose(
                            trp[:, TS * t:TS * (t + 1)],
                            et[:, t, 128 * k:128 * (k + 1)], ident_bf)
                    nc.vector.tensor_copy(encT[:, k, :L], trp[:, :L])
                return encT

            def compute(j, encT):
                nt = tcnt[j]
                L = nt * TS
                sig = sig_pool.tile([128, 4, S], BF16, tag="sig", name="sig")
                for m in range(4):
                    eps_t = mm_ps.tile([128, S], F32, tag="mm", name="eps_t")
                    for k in range(4):
                        nc.tensor.matmul(
                            eps_t[:, :L], WeT_sb[:, k, 128 * m:128 * (m + 1)],
                            encT[:, k, :L], start=(k == 0), stop=(k == 3))
                    nc.scalar.activation(
                        out=sig[:, m, :L], in_=eps_t[:, :L],
                        func=AF.Sigmoid, bias=hid_sb[:, m, j:j + 1])
                erow = row_ps.tile([1, S], F32, tag="row", name="erow")
                for m in range(4):
                    nc.tensor.matmul(erow[:, :L], Wv_sb[:, m:m + 1],
                                     sig[:, m, :L],
                                     start=(m == 0), stop=(m == 3))
                etmp = rows.tile([1, S], F32, tag="etmp", name="etmp")
                nc.vector.tensor_copy(etmp[:, :L], erow[:, :L])
                nc.gpsimd.dma_start(out=energy_sb[j:j + 1, :L],
                                    in_=etmp[:, :L],
                                    accum_op=mybir.AluOpType.add)


            # ---------------- GRU ----------------
            # phase A: gi = W_ih @ [palette; last_context].T
            nc.sync.dma_start(out=WeT_sb, in_=ktiled(WeT, 4, H, 2))
            _load0(0)
            wih = gruw.tile([128, 4, 3 * H], BF16, tag="gw", name="wih")
            nc.sync.dma_start(out=wih, in_=ktiled(WihCT, 4, 3 * H, 2))
            _load0(1)
            whh = gruw.tile([128, 4, 3 * H], BF16, tag="gw", name="whh")
            nc.sync.dma_start(out=whh, in_=ktiled(WhhT, 4, 3 * H, 2))
            nc.sync.dma_start(out=WhT_sb, in_=ktiled(WhT, 4, H, 4))
            _load0(2)
            _load0(3)
            encTs[0] = transpose(0)
            for mo in range(12):
                ps = mm_ps.tile([128, BC], F32, tag="mm", name="ps_gi")
                for k in range(4):
                    nc.tensor.matmul(ps, wih[:, k, 128 * mo:128 * (mo + 1)],
                                     lcTb_sb[:, k, :], start=(k == 0), stop=False)
                nc.tensor.matmul(ps, WihPT_sb[:, 128 * mo:128 * (mo + 1)],
                                 palT_sb, start=False, stop=True)
                nc.scalar.copy(gi_sb[:, mo, :], ps)
            encTs[1] = transpose(1)
            # phase B: gh = W_hh @ h_prev.T ; gates
            rz = []
            for mo in range(8):
                ps = mm_ps.tile([128, BC], F32, tag="mm", name="ps_gh")
                for k in range(4):
                    nc.tensor.matmul(ps, whh[:, k, 128 * mo:128 * (mo + 1)],
                                     lhTb_sb[:, k, :], start=(k == 0), stop=(k == 3))
                s_t = small.tile([128, BC], F32, tag=f"rzs{mo}", bufs=1,
                                 name=f"rzs{mo}")
                nc.vector.tensor_add(s_t, ps, gi_sb[:, mo, :])
                dst = small.tile([128, BC], F32, tag=f"rz{mo}", bufs=1,
                                 name=f"rz{mo}")
                nc.scalar.activation(out=dst, in_=s_t, func=AF.Sigmoid,
                                     bias=brz_sb[:, mo:mo + 1])
                rz.append(dst)
            for m in range(4):
                mo = 8 + m
                ps_h = mm_ps.tile([128, BC], F32, tag="mm", name="ps_hn")
                for k in range(4):
                    nc.tensor.matmul(ps_h, whh[:, k, 128 * mo:128 * (mo + 1)],
                                     lhTb_sb[:, k, :], start=(k == 0), stop=(k == 3))
                i_n = small.tile([128, BC], F32, tag="i_n", name="i_n")
                nc.scalar.activation(out=i_n, in_=gi_sb[:, mo, :],
                                     func=AF.Identity, bias=bin_sb[:, m:m + 1])
                h_n = small.tile([128, BC], F32, tag="h_n", name="h_n")
                nc.scalar.activation(out=h_n, in_=ps_h, func=AF.Identity,
                                     bias=bhn_sb[:, m:m + 1])
                tmp = small.tile([128, BC], F32, tag="tmp", name="tmp")
                nc.vector.tensor_mul(tmp, rz[m], h_n)
                nc.vector.tensor_add(tmp, tmp, i_n)
                n_t = small.tile([128, BC], F32, tag="n_t", name="n_t")
                nc.scalar.activation(out=n_t, in_=tmp, func=AF.Tanh)
                d_t = small.tile([128, BC], F32, tag="d_t", name="d_t")
                nc.vector.tensor_sub(d_t, lhT_sb[:, m, :], n_t)
                nc.vector.tensor_mul(d_t, d_t, rz[4 + m])
                nc.vector.tensor_add(h_sb[:, m, :], n_t, d_t)
            nc.sync.dma_start(
                out=bass.AP(tensor=hT_out, offset=0,
                            ap=[[BC, 128], [128 * BC, 4], [1, BC]]),
                in_=h_sb)
            # W1 weights: prefetch into the gruw slot (overlaps attention)
            w1 = gruw.tile([128, 8, H], F32, tag="gw", name="w1")
            nc.sync.dma_start(out=w1, in_=ktiled(W1T, 8, H, 4))
            # hid = Wh @ h_new.T + (be + bh)
            for m in range(4):
                ps = mm_ps.tile([128, BC], F32, tag="mm", name="ps_hid")
                for k in range(4):
                    nc.tensor.matmul(ps, WhT_sb[:, k, 128 * m:128 * (m + 1)],
                                     h_sb[:, k, :], start=(k == 0), stop=(k == 3))
                nc.scalar.activation(out=hid_sb[:, m, :], in_=ps,
                                     func=AF.Identity, bias=bebh_sb[:, m:m + 1])

            # ---------------- attention ----------------
            # energy rows start as the additive mask; device adds Wv.sig via
            # accumulating scatter-DMAs
            nc.sync.dma_start(out=energy_sb, in_=maskadd.ap())

            load = _load0

            for j in range(2, BC):
                if j + 2 < BC:
                    load(j + 2)
                encTs[j] = transpose(j)
                compute(j - 2, encTs.pop(j - 2))
            compute(BC - 2, encTs.pop(BC - 2))
            compute(BC - 1, encTs.pop(BC - 1))

            # masked softmax over all 32 rows
            nc.vector.tensor_reduce(
                out=stat_sb[:, 0:1], in_=energy_sb, axis=mybir.AxisListType.X,
                op=mybir.AluOpType.max, negate=True)
            nc.scalar.activation(
                out=aw_sb, in_=energy_sb, func=AF.Exp,
                bias=stat_sb[:, 0:1], accum_out=stat_sb[:, 1:2])
            nc.vector.reciprocal(out=stat_sb[:, 2:3], in_=stat_sb[:, 1:2])
            awbf = aw_pool.tile([BC, S], BF16, tag="awbf", name="awbf")
            awT = aw_pool.tile([128, 4, BC], BF16, tag="awT", name="awT")
            for t in range(4):
                tb = slice(TS * t, TS * (t + 1))
                nc.vector.tensor_scalar_mul(aw_sb[:, tb], aw_sb[:, tb],
                                            stat_sb[:, 2:3])
                nc.vector.tensor_copy(awbf[:, tb], aw_sb[:, tb])
                awt_p = awt_ps_pool.tile([128, BC], BF16, tag="trp",
                                         name="awt_p")
                nc.tensor.transpose(awt_p, awbf[:, tb], ident_bf[0:BC, 0:BC])
                nc.scalar.copy(awT[:, t, :], awt_p)
            CCH = 4
            ctx_dram = dram.tile([BC, H], F32, tag="ctxd", name="ctx_dram")
            ctps = []
            for c in range(4):
                ctp = awt_ps_pool.tile([128, BC], F32, tag="trp", name="ctp")
                ctps.append(ctp)
            for j in range(BC):
                nt = tcnt[j]
                crow = row_ps.tile([1, H], F32, tag="row", name="crow")
                for t in range(nt):
                    nc.tensor.matmul(
                        crow, awT[:, t, j:j + 1],
                        enc_tiles[j][:, t, :], start=(t == 0),
                        stop=(t == nt - 1))
                if j % CCH == 0:
                    cch = rows.tile([1, CCH, H], F32, tag="cch", bufs=2,
                                    name="cch")
                nc.vector.tensor_copy(cch[:, j % CCH, :], crow)
                if j % CCH == CCH - 1:
                    g = j // CCH
                    nc.sync.dma_start(
                        out=ctx_dram[j - CCH + 1:j + 1, :], in_=cch)
                    chg = rows.tile([CCH, H], F32, tag="chg", bufs=2,
                                    name="chg")
                    nc.sync.dma_start(
                        out=chg, in_=ctx_dram[j - CCH + 1:j + 1, :])
                    for c in range(4):
                        nc.tensor.transpose(
                            ctps[c][:, CCH * g:CCH * (g + 1)],
                            chg[:, 128 * c:128 * (c + 1)],
                            ident_f[0:CCH, 0:CCH])
            for c in range(4):
                nc.vector.tensor_copy(ctxT_sb[:, c, :], ctps[c])
            nc.sync.dma_start(out=ctx_out.ap(), in_=ctx_dram)

            nc.sync.dma_start(out=aw_out.ap(), in_=aw_sb)

            # ---------------- output MLP (pre-BN) ----------------
            for m in range(4):
                ps = mm_ps.tile([128, BC], F32, tag="mm", name="ps_y")
                for k in range(4):
                    nc.tensor.matmul(ps, w1[:, k, 128 * m:128 * (m + 1)],
                                     h_sb[:, k, :], start=(k == 0), stop=False)
                for k in range(4):
                    nc.tensor.matmul(ps, w1[:, 4 + k, 128 * m:128 * (m + 1)],
                                     ctxT_sb[:, k, :], start=False, stop=(k == 3))
                nc.scalar.activation(out=y_sb[:, m, :], in_=ps, func=AF.Relu,
                                     bias=b1_sb[:, m:m + 1])
            nc.sync.dma_start(
                out=bass.AP(tensor=yT_out, offset=0,
                            ap=[[BC, 128], [128 * BC, 4], [1, BC]]),
                in_=y_sb)

        if reps == 1:
            body()
        else:
            with tc.For_i(0, reps, 1):
                body()

    nc.compile()
    nc.finalize()
    return nc


# ----------------------------------------------------------------------------
# Host glue
# ----------------------------------------------------------------------------

def _as_f32(x):
    return np.ascontiguousarray(np.asarray(x), dtype=np.float32)


def prepare(inputs):
    """Host-side planning + per-core input maps."""
    palette = _as_f32(inputs["palette"])
    last_context = _as_f32(inputs["last_context"])
    last_hidden = _as_f32(inputs["last_hidden"])
    enc = _as_f32(inputs["encoder_outputs"])
    lengths = np.asarray(inputs["each_input_size"]).astype(np.int64)

    perm, tcnt = _plan(lengths)

    We = _as_f32(inputs["We"]); be = _as_f32(inputs["be"])
    Wh = _as_f32(inputs["Wh"]); bh = _as_f32(inputs["bh"])
    Wv = _as_f32(inputs["Wv"])
    W_ih = _as_f32(inputs["W_ih"]); W_hh = _as_f32(inputs["W_hh"])
    b_ih = _as_f32(inputs["b_ih"]); b_hh = _as_f32(inputs["b_hh"])
    W1 = _as_f32(inputs["W1"]); b1 = _as_f32(inputs["b1"])

    WeT = np.ascontiguousarray(We.T).astype(bf16)
    Wv4 = np.ascontiguousarray(Wv[0].reshape(4, 128).T).astype(bf16)
    WihPT = np.ascontiguousarray(W_ih[:, :P].T).astype(bf16)
    WihCT = np.ascontiguousarray(W_ih[:, P:].T).astype(bf16)
    WhhT = np.ascontiguousarray(W_hh.T).astype(bf16)
    WhT = np.ascontiguousarray(Wh.T)
    W1T = np.ascontiguousarray(W1.T)
    brz_t = (b_ih + b_hh)[:1024].reshape(8, 128).T
    bin_t = b_ih[1024:].reshape(4, 128).T
    bhn_t = b_hh[1024:].reshape(4, 128).T
    bebh_t = (be + bh).reshape(4, 128).T
    b1_t = b1.reshape(4, 128).T
    biases = np.ascontiguousarray(
        np.concatenate([brz_t, bin_t, bhn_t, bebh_t, b1_t], axis=1))

    palT_full = np.ascontiguousarray(palette[0].T)
    lcT_full = np.ascontiguousarray(last_context[0].T)
    lhT_full = np.ascontiguousarray(last_hidden[0].T)

    in_maps = []
    for c in range(NCORES):
        idx = np.asarray(perm[c])
        maskadd = np.full((BC, S), NEG, np.float32)
        for slot, bi in enumerate(idx):
            maskadd[slot, :lengths[bi]] = 0.0
        m = {
            "enc": np.ascontiguousarray(enc[:, idx, :]).astype(bf16),
            "maskadd": maskadd,
            "palT": np.ascontiguousarray(palT_full[:, idx]).astype(bf16),
            "lcTb": np.ascontiguousarray(lcT_full[:, idx]).astype(bf16),
            "lhTb": np.ascontiguousarray(lhT_full[:, idx]).astype(bf16),
            "lhT": np.ascontiguousarray(lhT_full[:, idx]),
            "WeT": WeT, "Wv4": Wv4,
            "WihPT": WihPT, "WihCT": WihCT, "WhhT": WhhT, "WhT": WhT,
            "W1T": W1T, "biases": biases,
        }
        in_maps.append(m)
    return in_maps, perm, tcnt


def finish(results, perm, inputs):
    """Gather per-core outputs, apply BatchNorm + final linear on host."""
    gamma = _as_f32(inputs["gamma"]); beta = _as_f32(inputs["beta"])
    W2 = _as_f32(inputs["W2"]); b2 = _as_f32(inputs["b2"])

    y = np.zeros((B, H), np.float32)
    context = np.zeros((1, B, H), np.float32)
    hidden = np.zeros((1, B, H), np.float32)
    attn = np.zeros((B, 1, S), np.float32)
    for c in range(NCORES):
        idx = np.asarray(perm[c])
        r = results[c]
        y[idx] = np.asarray(r["yT_out"]).T
        context[0, idx] = np.asarray(r["ctx_out"])
        hidden[0, idx] = np.asarray(r["hT_out"]).T
        attn[idx, 0, :] = np.asarray(r["aw_out"])

    mu = y.mean(axis=0)
    var = y.var(axis=0)
    yn = (y - mu) * (1.0 / np.sqrt(var + EPS)) * gamma + beta
    out = yn @ W2.T + b2
    return (out.astype(np.float32), context, hidden, attn)


_prog_cache = {}


def get_program(tcnt, reps=1):
    key = (tuple(tcnt), reps)
    if key not in _prog_cache:
        _prog_cache[key] = build_program(list(tcnt), reps=reps)
    return _prog_cache[key]


# ----------------------------------------------------------------------------
# Cached PJRT runner (avoids re-trace / re-compile on repeated dispatch)
# ----------------------------------------------------------------------------

import jax
from jax.sharding import Mesh, PartitionSpec, NamedSharding
from jax.experimental.shard_map import shard_map
from concourse import bass2jax

try:
    jax.config.update("jax_compilation_cache_dir", "/tmp/jax_cc_cache")
    jax.config.update("jax_persistent_cache_min_compile_time_secs", 1.0)
    jax.config.update("jax_persistent_cache_min_entry_size_bytes", 0)
except Exception:
    pass


class Runner:
    def __init__(self, nc):
        bass2jax.install_neuronx_cc_hook()
        self.nc = nc
        partition_name = (nc.partition_id_tensor.name
                          if nc.partition_id_tensor else None)
        in_names, out_names, out_avals = [], [], []
        for alloc in nc.m.functions[0].allocations:
            if not isinstance(alloc, mybir.MemoryLocationSet):
                continue
            name = alloc.memorylocations[0].name
            if alloc.kind == "ExternalInput":
                if name != partition_name:
                    in_names.append(name)
            elif alloc.kind == "ExternalOutput":
                out_names.append(name)
                out_avals.append(jax.core.ShapedArray(
                    tuple(alloc.tensor_shape), mybir.dt.np(alloc.dtype)))
        self.in_names = in_names
        self.out_names = out_names
        self.out_avals = out_avals
        n_params = len(in_names)
        all_in_names = in_names + out_names + (
            [partition_name] if partition_name else [])
        donate = tuple(range(n_params, n_params + len(out_names)))

        def _body(*args):
            operands = list(args)
            if partition_name is not None:
                operands.append(bass2jax.partition_id_tensor())
            outs = bass2jax._bass_exec_p.bind(
                *operands,
                out_avals=tuple(out_avals),
                in_names=tuple(all_in_names),
                out_names=tuple(out_names),
                lowering_input_output_aliases=(),
                sim_require_finite=True,
                sim_require_nnan=True,
                nc=nc,
            )
            return tuple(outs)

        devices = jax.devices()[:NCORES]
        self.mesh = Mesh(np.asarray(devices), ("core",))
        in_specs = (PartitionSpec("core"),) * (n_params + len(out_names))
        out_specs = (PartitionSpec("core"),) * len(out_names)
        self.sharded = jax.jit(
            shard_map(_body, mesh=self.mesh, in_specs=in_specs,
                      out_specs=out_specs, check_rep=False),
            donate_argnums=donate, keep_unused=True)
        self.sharding = NamedSharding(self.mesh, PartitionSpec("core"))

    def put(self, in_maps):
        concat = [
            np.concatenate([np.asarray(in_maps[c][n]) for c in range(NCORES)],
                           axis=0)
            for n in self.in_names
        ]
        return [jax.device_put(a, self.sharding) for a in concat]

    def zeros(self):
        return [np.zeros((NCORES * av.shape[0], *av.shape[1:]), av.dtype)
                for av in self.out_avals]

    def call(self, dev_in):
        outs = self.sharded(*dev_in, *self.zeros())
        jax.block_until_ready(outs)
        return outs

    def results(self, outs):
        return [
            {name: np.asarray(outs[i]).reshape(
                NCORES, *self.out_avals[i].shape)[c]
             for i, name in enumerate(self.out_names)}
            for c in range(NCORES)
        ]


_runner_cache = {}


def get_runner(tcnt, reps=1):
    key = (tuple(tcnt), reps)
    if key not in _runner_cache:
        _runner_cache[key] = Runner(get_program(tcnt, reps=reps))
    return _runner_cache[key]


def kernel(**inputs):
    in_maps, perm, tcnt = prepare(inputs)
    r = get_runner(tcnt)
    outs = r.call(r.put(in_maps))
    return finish(r.results(outs), perm, inputs)


# revision 23
# speedup vs baseline: 1.0345x; 1.0345x over previous
"""Trainium2 Bass kernel for nn_AttnDecoderRNN (GRU step + Bahdanau attention
over ragged sequences + BatchNorm MLP head).

Self-contained: hardcodes shapes (H=512, B=256, S=512, P=3) and the
data-parallel sharding (batch dim split across 8 cores).

Strategy
--------
Batch-parallel over 8 cores (32 batches each). Host bin-packs batches so every
core receives the same multiset of per-batch s-tile counts (ceil(len/128),
with a few batches promoted so class counts divide by 8); the device program
is compiled specialized to that static schedule, so invalid s-tiles are never
loaded or computed.

Per core: GRU runs in transposed [H, batch] layout on PE. Per batch: enc tiles
are DMA-cast fp32->bf16, PE-transposed to put H on partitions, multiplied with
We.T in bf16, sigmoid applied on ACT with the per-batch hidden projection as a
per-partition bias, reduced against Wv by matmul, then a grouped masked
softmax (fused exp+row-sum) and a context matmul against the natural-layout
enc tiles. The output MLP (W1+ReLU) runs on-device; BatchNorm statistics
(which couple all 256 batches) and the tiny 512->3 output matmul finish on
host.
"""

import sys
from contextlib import ExitStack

sys.path.insert(0, "/opt/trn_rl_repo")

import numpy as np
import ml_dtypes

import concourse.bacc as bacc
import concourse.tile as tile
from concourse import mybir
from concourse.masks import make_identity
from concourse.bass_utils import run_bass_kernel_spmd

F32 = mybir.dt.float32
FP8 = mybir.dt.float8e4
BF16 = mybir.dt.bfloat16
AF = mybir.ActivationFunctionType

H = 512
B = 256
S = 512
P = 3
EPS = 1e-5
NCORES = 8
BC = B // NCORES          # 32 batches per core
TS = 128                  # s-tile size
NT_MAX = S // TS          # 4 tiles max per batch
GROUP = 8                 # batches per softmax group
NEG = -30000.0            # additive mask value (exp -> exact 0)

bf16 = ml_dtypes.bfloat16


# ----------------------------------------------------------------------------
# Host-side planning
# ----------------------------------------------------------------------------

def _plan(lengths: np.ndarray):
    """Assign batches to cores so each core gets an identical multiset of
    s-tile counts. Returns (perm[8][32] original batch idx, tcnt[32])."""
    lengths = np.asarray(lengths).astype(np.int64)
    cls = np.clip((lengths + TS - 1) // TS, 1, NT_MAX).astype(np.int64)
    # promote batches so each class count is divisible by NCORES
    by_cls = {k: [int(i) for i in np.nonzero(cls == k)[0]] for k in range(1, 5)}
    for k in range(1, 4):
        r = len(by_cls[k]) % NCORES
        if r:
            moved = sorted(by_cls[k], key=lambda i: lengths[i])[-r:]
            for i in moved:
                by_cls[k].remove(i)
                by_cls[k + 1].append(i)
    assert len(by_cls[4]) % NCORES == 0
    per_core_cls = {k: len(by_cls[k]) // NCORES for k in range(1, 5)}

    # per-core slot schedule: deal classes (desc) round-robin into the softmax
    # groups so per-group tile counts stay balanced
    tcnt = []
    for k in (1, 2, 3, 4):
        tcnt += [k] * per_core_cls[k]
    assert len(tcnt) == BC

    perm = [[None] * BC for _ in range(NCORES)]
    for c in range(NCORES):
        chunk = {k: by_cls[k][c * per_core_cls[k]:(c + 1) * per_core_cls[k]]
                 for k in range(1, 5)}
        ptr = {k: 0 for k in range(1, 5)}
        for slot, k in enumerate(tcnt):
            perm[c][slot] = chunk[k][ptr[k]]
            ptr[k] += 1
    return perm, tcnt


# ----------------------------------------------------------------------------
# Device program
# ----------------------------------------------------------------------------

def build_program(tcnt, reps: int = 1):
    """Build + compile the per-core Bass program for the given (static)
    per-slot tile counts. Returns finalized nc."""
    import concourse.bass as bass

    nc = bacc.Bacc("TRN2", target_bir_lowering=False, debug=False,
                   num_devices=NCORES)

    def din(name, shape, dt):
        return nc.dram_tensor(name, shape, dt, kind="ExternalInput")

    def dout(name, shape, dt):
        return nc.dram_tensor(name, shape, dt, kind="ExternalOutput")

    enc = din("enc", [S, BC, H], BF16)
    maskadd = din("maskadd", [BC, S], F32)
    bfblob = din("bfblob", [128, 1828], BF16)
    fblob = din("fblob", [128, 152], F32)
    WeT = din("WeT", [H, H], FP8)
    Wv4 = din("Wv4", [128, 4], BF16)
    WihCT = din("WihCT", [H, 3 * H], BF16)
    WhhT = din("WhhT", [H, 3 * H], BF16)
    WhT = din("WhT", [H, H], F32)
    W1T = din("W1T", [2 * H, H], F32)

    hT_out = dout("hT_out", [H, BC], F32)
    ctx_out = dout("ctx_out", [BC, H], F32)
    yT_out = dout("yT_out", [H, BC], F32)
    aw_out = dout("aw_out", [BC, S], F32)

    NT = sum(tcnt)
    n_cls = {k: sum(1 for c in tcnt if c == k) for k in range(1, 5)}

    def ktiled(handle, nk, width, dt_size):
        """DRAM AP view [128, nk, width] of a [128*nk, width] tensor."""
        return bass.AP(
            tensor=handle, offset=0,
            ap=[[width, 128], [128 * width, nk], [1, width]])

    with tile.TileContext(nc) as tc, ExitStack() as ctx:
        consts = ctx.enter_context(tc.tile_pool(name="consts", bufs=1))

        ident_bf = consts.tile([128, 128], BF16)
        make_identity(nc, ident_bf)
        ident_f = consts.tile([128, 128], F32)
        make_identity(nc, ident_f)

        WeT_sb = consts.tile([128, 4, H], FP8)
        WhT_sb = consts.tile([128, 4, H], F32)
        bfb = consts.tile([128, 1828], BF16)
        fb = consts.tile([128, 152], F32)
        Wv_sb = bfb[:, 0:4]
        lcTb_sb = bfb[:, 4:132].rearrange("p (k j) -> p k j", k=4)
        lhTb_sb = bfb[:, 132:260].rearrange("p (k j) -> p k j", k=4)
        palT_sb = bfb[0:P, 260:292]
        WihPT_sb = bfb[0:P, 292:1828]
        brz_sb = fb[:, 0:8]
        bin_sb = fb[:, 8:12]
        bhn_sb = fb[:, 12:16]
        bebh_sb = fb[:, 16:20]
        b1_sb = fb[:, 20:24]
        lhT_sb = fb[:, 24:152].rearrange("p (k j) -> p k j", k=4)

        persist = ctx.enter_context(tc.tile_pool(name="persist", bufs=1))
        h_sb = persist.tile([128, 4, BC], F32)        # h_new.T
        hid_sb = persist.tile([128, 4, BC], F32)      # Wh@h_new + be + bh
        gi_sb = persist.tile([128, 12, BC], F32)      # W_ih @ x.T
        energy_sb = persist.tile([BC, S], F32)
        aw_sb = persist.tile([BC, S], F32)
        ctxT_sb = persist.tile([128, 4, BC], F32)
        y_sb = persist.tile([128, 4, BC], F32)
        stat_sb = persist.tile([BC, 3], F32)          # negmax | sumexp | recip

        # PSUM: mm (2) + tr (2) + row (2) + awt (2) = 8 banks
        mm_ps = ctx.enter_context(tc.tile_pool(name="mm_ps", bufs=3, space="PSUM"))
        tr_ps = ctx.enter_context(tc.tile_pool(name="tr_ps", bufs=3, space="PSUM"))
        row_ps = ctx.enter_context(tc.tile_pool(name="row_ps", bufs=2, space="PSUM"))
        awt_ps_pool = tr_ps

        small = ctx.enter_context(tc.tile_pool(name="small", bufs=4))
        # one 24KB/partition slot reused: WihCT -> WhhT -> W1T
        gruw = ctx.enter_context(tc.tile_pool(name="gruw", bufs=2))
        enc_pool = ctx.enter_context(tc.tile_pool(name="enc_pool", bufs=1))
        encT_pool = ctx.enter_context(tc.tile_pool(name="encT_pool", bufs=4))
        sig_pool = ctx.enter_context(tc.tile_pool(name="sig_pool", bufs=3))
        aw_pool = ctx.enter_context(tc.tile_pool(name="aw_pool", bufs=2))
        rows = ctx.enter_context(tc.tile_pool(name="rows", bufs=4))
        dram = ctx.enter_context(tc.tile_pool(name="dram", bufs=1, space="DRAM"))

        def enc_batch_ap(j, nt):
            # [128 p(s%128), nt t(s//128), H] view of enc[:, j, :]
            return bass.AP(
                tensor=enc, offset=j * H,
                ap=[[BC * H, 128], [TS * BC * H, nt], [1, H]])

        def body(_iv=None):
            PRE = 4
            enc_tiles = {}
            encTs = {}

            def _load0(j):
                nt = tcnt[j]
                et = enc_pool.tile([128, nt, H], BF16, tag=f"enc{nt}",
                                   bufs=n_cls[nt], name="et")
                nc.sync.dma_start(out=et, in_=enc_batch_ap(j, nt))
                enc_tiles[j] = et

            def transpose(j):
                nt = tcnt[j]
                L = nt * TS
                et = enc_tiles[j]
                encT = encT_pool.tile([128, 4, S], FP8, tag="encT",
                                      name="encT")
                for k in range(4):
                    trp = tr_ps.tile([128, S], BF16, tag="trp", name="trp")
                    for t in range(nt):
                        nc.tensor.transpose(
                            trp[:, TS * t:TS * (t + 1)],
                            et[:, t, 128 * k:128 * (k + 1)], ident_bf)
                    nc.vector.tensor_copy(encT[:, k, :L], trp[:, :L])
                return encT

            def compute(j, encT, pre=None):
                nt = tcnt[j]
                L = nt * TS
                sig = sig_pool.tile([128, 4, S], BF16, tag="sig", name="sig")
                for m in range(4):
                    if pre is not None and m in pre:
                        eps_t = pre[m]
                    else:
                        eps_t = mm_ps.tile([128, S], F32, tag="mm",
                                           name="eps_t")
                        for kp in (0, 2):
                            nc.tensor.matmul(
                                eps_t[:, :L],
                                WeT_sb[:, kp:kp + 2, 128 * m:128 * (m + 1)],
                                encT[:, kp:kp + 2, :L],
                                start=(kp == 0), stop=(kp == 2),
                                perf_mode=mybir.MatmulPerfMode.DoubleRow)
                    nc.scalar.activation(
                        out=sig[:, m, :L], in_=eps_t[:, :L],
                        func=AF.Sigmoid, bias=hid_sb[:, m, j:j + 1])
                erow = row_ps.tile([1, S], F32, tag="row", name="erow")
                for m in range(4):
                    nc.tensor.matmul(erow[:, :L], Wv_sb[:, m:m + 1],
                                     sig[:, m, :L],
                                     start=(m == 0), stop=(m == 3))
                etmp = rows.tile([1, S], F32, tag="etmp", name="etmp")
                nc.vector.tensor_copy(etmp[:, :L], erow[:, :L])
                nc.gpsimd.dma_start(out=energy_sb[j:j + 1, :L],
                                    in_=etmp[:, :L],
                                    accum_op=mybir.AluOpType.add)


            # ---------------- GRU ----------------
            # phase A: gi = W_ih @ [palette; last_context].T
            nc.sync.dma_start(out=WeT_sb, in_=ktiled(WeT, 4, H, 1))
            _load0(0)
            nc.sync.dma_start(out=bfb, in_=bfblob.ap())
            wih = gruw.tile([128, 4, 3 * H], BF16, tag="gw", name="wih")
            nc.sync.dma_start(out=wih, in_=ktiled(WihCT, 4, 3 * H, 2))
            _load0(1)
            nc.sync.dma_start(out=fb, in_=fblob.ap())
            whh = gruw.tile([128, 4, 3 * H], BF16, tag="gw", name="whh")
            nc.sync.dma_start(out=whh, in_=ktiled(WhhT, 4, 3 * H, 2))
            nc.sync.dma_start(out=WhT_sb, in_=ktiled(WhT, 4, H, 4))
            _load0(2)
            _load0(3)
            encTs[0] = transpose(0)
            encTs[1] = transpose(1)
            encTs[2] = transpose(2)
            encTs[3] = transpose(3)
            # E-matmul prefix for batch 0 — fills the GRU weight wait
            pre_eps = {}
            L0 = tcnt[0] * TS
            for m in range(3):
                eps_t = mm_ps.tile([128, S], F32, tag="mm", name="eps_t")
                for kp in (0, 2):
                    nc.tensor.matmul(
                        eps_t[:, :L0],
                        WeT_sb[:, kp:kp + 2, 128 * m:128 * (m + 1)],
                        encTs[0][:, kp:kp + 2, :L0],
                        start=(kp == 0), stop=(kp == 2),
                        perf_mode=mybir.MatmulPerfMode.DoubleRow)
                pre_eps[m] = eps_t
            for mo in range(12):
                ps = row_ps.tile([128, BC], F32, tag="row", name="ps_gi")
                for k in range(4):
                    nc.tensor.matmul(ps, wih[:, k, 128 * mo:128 * (mo + 1)],
                                     lcTb_sb[:, k, :], start=(k == 0), stop=False)
                nc.tensor.matmul(ps, WihPT_sb[:, 128 * mo:128 * (mo + 1)],
                                 palT_sb, start=False, stop=True)
                nc.scalar.copy(gi_sb[:, mo, :], ps)
            # phase B: gh = W_hh @ h_prev.T ; gates
            rz = []
            for mo in range(8):
                ps = row_ps.tile([128, BC], F32, tag="row", name="ps_gh")
                for k in range(4):
                    nc.tensor.matmul(ps, whh[:, k, 128 * mo:128 * (mo + 1)],
                                     lhTb_sb[:, k, :], start=(k == 0), stop=(k == 3))
                s_t = small.tile([128, BC], F32, tag=f"rzs{mo}", bufs=1,
                                 name=f"rzs{mo}")
                nc.vector.tensor_add(s_t, ps, gi_sb[:, mo, :])
                dst = small.tile([128, BC], F32, tag=f"rz{mo}", bufs=1,
                                 name=f"rz{mo}")
                nc.scalar.activation(out=dst, in_=s_t, func=AF.Sigmoid,
                                     bias=brz_sb[:, mo:mo + 1])
                rz.append(dst)
            for m in range(4):
                mo = 8 + m
                ps_h = row_ps.tile([128, BC], F32, tag="row", name="ps_hn")
                for k in range(4):
                    nc.tensor.matmul(ps_h, whh[:, k, 128 * mo:128 * (mo + 1)],
                                     lhTb_sb[:, k, :], start=(k == 0), stop=(k == 3))
                i_n = small.tile([128, BC], F32, tag="i_n", name="i_n")
                nc.scalar.activation(out=i_n, in_=gi_sb[:, mo, :],
                                     func=AF.Identity, bias=bin_sb[:, m:m + 1])
                h_n = small.tile([128, BC], F32, tag="h_n", name="h_n")
                nc.scalar.activation(out=h_n, in_=ps_h, func=AF.Identity,
                                     bias=bhn_sb[:, m:m + 1])
                tmp = small.tile([128, BC], F32, tag="tmp", name="tmp")
                nc.vector.tensor_mul(tmp, rz[m], h_n)
                nc.vector.tensor_add(tmp, tmp, i_n)
                n_t = small.tile([128, BC], F32, tag="n_t", name="n_t")
                nc.scalar.activation(out=n_t, in_=tmp, func=AF.Tanh)
                d_t = small.tile([128, BC], F32, tag="d_t", name="d_t")
                nc.vector.tensor_sub(d_t, lhT_sb[:, m, :], n_t)
                nc.vector.tensor_mul(d_t, d_t, rz[4 + m])
                nc.vector.tensor_add(h_sb[:, m, :], n_t, d_t)
            nc.sync.dma_start(
                out=bass.AP(tensor=hT_out, offset=0,
                            ap=[[BC, 128], [128 * BC, 4], [1, BC]]),
                in_=h_sb)
            # W1 weights: prefetch into the gruw slot (overlaps attention)
            w1 = gruw.tile([128, 8, H], F32, tag="gw", name="w1")
            nc.sync.dma_start(out=w1, in_=ktiled(W1T, 8, H, 4))
            # hid = Wh @ h_new.T + (be + bh)
            for m in range(4):
                ps = row_ps.tile([128, BC], F32, tag="row", name="ps_hid")
                for k in range(4):
                    nc.tensor.matmul(ps, WhT_sb[:, k, 128 * m:128 * (m + 1)],
                                     h_sb[:, k, :], start=(k == 0), stop=(k == 3))
                nc.scalar.activation(out=hid_sb[:, m, :], in_=ps,
                                     func=AF.Identity, bias=bebh_sb[:, m:m + 1])

            # ---------------- attention ----------------
            # energy rows start as the additive mask; device adds Wv.sig via
            # accumulating scatter-DMAs
            nc.sync.dma_start(out=energy_sb, in_=maskadd.ap())

            load = _load0

            compute(0, encTs.pop(0), pre=pre_eps)
            load(4)
            load(5)
            for j in range(4, BC):
                if j + 2 < BC:
                    load(j + 2)
                encTs[j] = transpose(j)
                compute(j - 3, encTs.pop(j - 3))
            compute(BC - 3, encTs.pop(BC - 3))
            compute(BC - 2, encTs.pop(BC - 2))
            compute(BC - 1, encTs.pop(BC - 1))

            # masked softmax over all 32 rows
            nc.vector.tensor_reduce(
                out=stat_sb[:, 0:1], in_=energy_sb, axis=mybir.AxisListType.X,
                op=mybir.AluOpType.max, negate=True)
            nc.scalar.activation(
                out=aw_sb, in_=energy_sb, func=AF.Exp,
                bias=stat_sb[:, 0:1], accum_out=stat_sb[:, 1:2])
            nc.vector.reciprocal(out=stat_sb[:, 2:3], in_=stat_sb[:, 1:2])
            awbf = aw_pool.tile([BC, S], BF16, tag="awbf", name="awbf")
            awT = aw_pool.tile([128, 4, BC], BF16, tag="awT", name="awT")
            for t in range(4):
                tb = slice(TS * t, TS * (t + 1))
                nc.vector.tensor_scalar_mul(aw_sb[:, tb], aw_sb[:, tb],
                                            stat_sb[:, 2:3])
                nc.vector.tensor_copy(awbf[:, tb], aw_sb[:, tb])
                awt_p = awt_ps_pool.tile([128, BC], BF16, tag="trp",
                                         name="awt_p")
                nc.tensor.transpose(awt_p, awbf[:, tb], ident_bf[0:BC, 0:BC])
                nc.scalar.copy(awT[:, t, :], awt_p)
            CCH = 4
            ctx_dram = dram.tile([BC, H], F32, tag="ctxd", name="ctx_dram")
            ctps_all = mm_ps.tile([128, 4, BC], F32, tag="mm",
                                  name="ctps_all")
            for j in range(BC):
                nt = tcnt[j]
                crow = row_ps.tile([1, H], F32, tag="row", name="crow")
                for t in range(nt):
                    nc.tensor.matmul(
                        crow, awT[:, t, j:j + 1],
                        enc_tiles[j][:, t, :], start=(t == 0),
                        stop=(t == nt - 1))
                if j % CCH == 0:
                    cch = rows.tile([1, CCH, H], F32, tag="cch", bufs=2,
                                    name="cch")
                nc.vector.tensor_copy(cch[:, j % CCH, :], crow)
                if j % CCH == CCH - 1:
                    g = j // CCH
                    nc.sync.dma_start(
                        out=ctx_dram[j - CCH + 1:j + 1, :], in_=cch)
                    chg = rows.tile([CCH, H], F32, tag="chg", bufs=2,
                                    name="chg")
                    nc.sync.dma_start(
                        out=chg, in_=ctx_dram[j - CCH + 1:j + 1, :])
                    for c in range(4):
                        nc.tensor.transpose(
                            ctps_all[:, c, CCH * g:CCH * (g + 1)],
                            chg[:, 128 * c:128 * (c + 1)],
                            ident_f[0:CCH, 0:CCH])
            nc.vector.tensor_copy(ctxT_sb, ctps_all)
            nc.sync.dma_start(out=ctx_out.ap(), in_=ctx_dram)

            nc.sync.dma_start(out=aw_out.ap(), in_=aw_sb)

            # ---------------- output MLP (pre-BN) ----------------
            for m in range(4):
                ps = mm_ps.tile([128, BC], F32, tag="mm", name="ps_y")
                for k in range(4):
                    nc.tensor.matmul(ps, w1[:, k, 128 * m:128 * (m + 1)],
                                     h_sb[:, k, :], start=(k == 0), stop=False)
                for k in range(4):
                    nc.tensor.matmul(ps, w1[:, 4 + k, 128 * m:128 * (m + 1)],
                                     ctxT_sb[:, k, :], start=False, stop=(k == 3))
                nc.scalar.activation(out=y_sb[:, m, :], in_=ps, func=AF.Relu,
                                     bias=b1_sb[:, m:m + 1])
            nc.sync.dma_start(
                out=bass.AP(tensor=yT_out, offset=0,
                            ap=[[BC, 128], [128 * BC, 4], [1, BC]]),
                in_=y_sb)

        if reps == 1:
            body()
        else:
            with tc.For_i(0, reps, 1):
                body()

    nc.compile()
    nc.finalize()
    return nc


# ----------------------------------------------------------------------------
# Host glue
# ----------------------------------------------------------------------------

def _as_f32(x):
    return np.ascontiguousarray(np.asarray(x), dtype=np.float32)


def prepare(inputs):
    """Host-side planning + per-core input maps."""
    palette = _as_f32(inputs["palette"])
    last_context = _as_f32(inputs["last_context"])
    last_hidden = _as_f32(inputs["last_hidden"])
    enc = _as_f32(inputs["encoder_outputs"])
    lengths = np.asarray(inputs["each_input_size"]).astype(np.int64)

    perm, tcnt = _plan(lengths)

    We = _as_f32(inputs["We"]); be = _as_f32(inputs["be"])
    Wh = _as_f32(inputs["Wh"]); bh = _as_f32(inputs["bh"])
    Wv = _as_f32(inputs["Wv"])
    W_ih = _as_f32(inputs["W_ih"]); W_hh = _as_f32(inputs["W_hh"])
    b_ih = _as_f32(inputs["b_ih"]); b_hh = _as_f32(inputs["b_hh"])
    W1 = _as_f32(inputs["W1"]); b1 = _as_f32(inputs["b1"])

    f8 = mybir.dt.np(mybir.dt.float8e4)
    WeT = np.ascontiguousarray(We.T).astype(f8)
    Wv4 = np.ascontiguousarray(Wv[0].reshape(4, 128).T).astype(bf16)
    WihPT = np.ascontiguousarray(W_ih[:, :P].T).astype(bf16)
    WihCT = np.ascontiguousarray(W_ih[:, P:].T).astype(bf16)
    WhhT = np.ascontiguousarray(W_hh.T).astype(bf16)
    WhT = np.ascontiguousarray(Wh.T)
    W1T = np.ascontiguousarray(W1.T)
    brz_t = (b_ih + b_hh)[:1024].reshape(8, 128).T
    bin_t = b_ih[1024:].reshape(4, 128).T
    bhn_t = b_hh[1024:].reshape(4, 128).T
    bebh_t = (be + bh).reshape(4, 128).T
    b1_t = b1.reshape(4, 128).T

    palT_full = np.ascontiguousarray(palette[0].T)
    lcT_full = np.ascontiguousarray(last_context[0].T)
    lhT_full = np.ascontiguousarray(last_hidden[0].T)

    def kmajor(a):
        # [512, BC] -> [128, 4*BC] with col 32k+j = a[128k+p, j]
        return a.reshape(4, 128, -1).transpose(1, 0, 2).reshape(128, -1)

    in_maps = []
    for c in range(NCORES):
        idx = np.asarray(perm[c])
        maskadd = np.full((BC, S), NEG, np.float32)
        for slot, bi in enumerate(idx):
            maskadd[slot, :lengths[bi]] = 0.0
        bfblob = np.zeros((128, 1828), bf16)
        bfblob[:, 0:4] = Wv4
        bfblob[:, 4:132] = kmajor(lcT_full[:, idx].astype(bf16))
        bfblob[:, 132:260] = kmajor(lhT_full[:, idx].astype(bf16))
        bfblob[0:P, 260:292] = palT_full[:, idx].astype(bf16)
        bfblob[0:P, 292:1828] = WihPT
        fblob = np.zeros((128, 152), np.float32)
        fblob[:, 0:8] = brz_t
        fblob[:, 8:12] = bin_t
        fblob[:, 12:16] = bhn_t
        fblob[:, 16:20] = bebh_t
        fblob[:, 20:24] = b1_t
        fblob[:, 24:152] = kmajor(lhT_full[:, idx])
        m = {
            "enc": np.ascontiguousarray(enc[:, idx, :]).astype(bf16),
            "maskadd": maskadd,
            "bfblob": bfblob,
            "fblob": fblob,
            "WeT": WeT, "Wv4": Wv4,
            "WihCT": WihCT, "WhhT": WhhT, "WhT": WhT,
            "W1T": W1T,
        }
        in_maps.append(m)
    return in_maps, perm, tcnt


def finish(results, perm, inputs):
    """Gather per-core outputs, apply BatchNorm + final linear on host."""
    gamma = _as_f32(inputs["gamma"]); beta = _as_f32(inputs["beta"])
    W2 = _as_f32(inputs["W2"]); b2 = _as_f32(inputs["b2"])

    y = np.zeros((B, H), np.float32)
    context = np.zeros((1, B, H), np.float32)
    hidden = np.zeros((1, B, H), np.float32)
    attn = np.zeros((B, 1, S), np.float32)
    for c in range(NCORES):
        idx = np.asarray(perm[c])
        r = results[c]
        y[idx] = np.asarray(r["yT_out"]).T
        context[0, idx] = np.asarray(r["ctx_out"])
        hidden[0, idx] = np.asarray(r["hT_out"]).T
        attn[idx, 0, :] = np.asarray(r["aw_out"])

    mu = y.mean(axis=0)
    var = y.var(axis=0)
    yn = (y - mu) * (1.0 / np.sqrt(var + EPS)) * gamma + beta
    out = yn @ W2.T + b2
    return (out.astype(np.float32), context, hidden, attn)


_prog_cache = {}


def get_program(tcnt, reps=1):
    key = (tuple(tcnt), reps)
    if key not in _prog_cache:
        _prog_cache[key] = build_program(list(tcnt), reps=reps)
    return _prog_cache[key]


# ----------------------------------------------------------------------------
# Cached PJRT runner (avoids re-trace / re-compile on repeated dispatch)
# ----------------------------------------------------------------------------

import jax
from jax.sharding import Mesh, PartitionSpec, NamedSharding
from jax.experimental.shard_map import shard_map
from concourse import bass2jax

try:
    jax.config.update("jax_compilation_cache_dir", "/tmp/jax_cc_cache")
    jax.config.update("jax_persistent_cache_min_compile_time_secs", 1.0)
    jax.config.update("jax_persistent_cache_min_entry_size_bytes", 0)
except Exception:
    pass


class Runner:
    def __init__(self, nc):
        bass2jax.install_neuronx_cc_hook()
        self.nc = nc
        partition_name = (nc.partition_id_tensor.name
                          if nc.partition_id_tensor else None)
        in_names, out_names, out_avals = [], [], []
        for alloc in nc.m.functions[0].allocations:
            if not isinstance(alloc, mybir.MemoryLocationSet):
                continue
            name = alloc.memorylocations[0].name
            if alloc.kind == "ExternalInput":
                if name != partition_name:
                    in_names.append(name)
            elif alloc.kind == "ExternalOutput":
                out_names.append(name)
                out_avals.append(jax.core.ShapedArray(
                    tuple(alloc.tensor_shape), mybir.dt.np(alloc.dtype)))
        self.in_names = in_names
        self.out_names = out_names
        self.out_avals = out_avals
        n_params = len(in_names)
        all_in_names = in_names + out_names + (
            [partition_name] if partition_name else [])
        donate = tuple(range(n_params, n_params + len(out_names)))

        def _body(*args):
            operands = list(args)
            if partition_name is not None:
                operands.append(bass2jax.partition_id_tensor())
            outs = bass2jax._bass_exec_p.bind(
                *operands,
                out_avals=tuple(out_avals),
                in_names=tuple(all_in_names),
                out_names=tuple(out_names),
                lowering_input_output_aliases=(),
                sim_require_finite=True,
                sim_require_nnan=True,
                nc=nc,
            )
            return tuple(outs)

        devices = jax.devices()[:NCORES]
        self.mesh = Mesh(np.asarray(devices), ("core",))
        in_specs = (PartitionSpec("core"),) * (n_params + len(out_names))
        out_specs = (PartitionSpec("core"),) * len(out_names)
        self.sharded = jax.jit(
            shard_map(_body, mesh=self.mesh, in_specs=in_specs,
                      out_specs=out_specs, check_rep=False),
            donate_argnums=donate, keep_unused=True)
        self.sharding = NamedSharding(self.mesh, PartitionSpec("core"))

    def put(self, in_maps):
        concat = [
            np.concatenate([np.asarray(in_maps[c][n]) for c in range(NCORES)],
                           axis=0)
            for n in self.in_names
        ]
        return [jax.device_put(a, self.sharding) for a in concat]

    def zeros(self):
        return [np.zeros((NCORES * av.shape[0], *av.shape[1:]), av.dtype)
                for av in self.out_avals]

    def call(self, dev_in):
        outs = self.sharded(*dev_in, *self.zeros())
        jax.block_until_ready(outs)
        return outs

    def results(self, outs):
        return [
            {name: np.asarray(outs[i]).reshape(
                NCORES, *self.out_avals[i].shape)[c]
             for i, name in enumerate(self.out_names)}
            for c in range(NCORES)
        ]


_runner_cache = {}


def get_runner(tcnt, reps=1):
    key = (tuple(tcnt), reps)
    if key not in _runner_cache:
        _runner_cache[key] = Runner(get_program(tcnt, reps=reps))
    return _runner_cache[key]


def kernel(**inputs):
    in_maps, perm, tcnt = prepare(inputs)
    r = get_runner(tcnt)
    outs = r.call(r.put(in_maps))
    return finish(r.results(outs), perm, inputs)
